# revision 2
# baseline (speedup 1.0000x reference)
"""Trainium2 Bass kernel for nn_Encoder_WordLstm (bi-LSTM over char/bichar embeddings).

Sharding: data-parallel over batch. Each of the 8 cores handles 8 sentences and
runs BOTH LSTM directions, merged into one set of [40, *] tensors (left chain on
partitions 0:8, right chain on 32:40) so each elementwise/activation instruction
covers both chains at once. Matmul operands are bf16 (fp32 PSUM accumulate).

The g-gate's pre-activation is scaled by 2 in the weights so sigmoid(2g) serves
as (tanh(g)+1)/2; the DVE algebra c' = f*c + 2*(i*s) - i recovers i*tanh(g)
without a separate tanh instruction.

Per-core pipeline (all on device):
  1. indirect-DMA gathers of 4 embedding streams x 2 sides -> feat [128tok, 800]
  2. PE transpose -> featT(bf16), matmul W_lin + tanh -> linT [300, 128tok] bf16
  3. matmul Wih (bias via ones-row augmentation) -> x tiles (bf16) -> DRAM
  4. 512-step LSTM recurrence; the right direction consumes pre-reversed gather
     indices so its scan is a plain forward loop; h fed back via PE transposes.
Output hs [2, 4096, 300] f32 per core; host reassembles [64, 512, 600].
"""

import os
import sys

import numpy as np

sys.path.insert(0, "/opt/trn_rl_repo")

import concourse.bass as bass
import concourse.bacc as bacc
import concourse.mybir as mybir
import concourse.tile as tile
from concourse.bass_utils import run_bass_kernel_spmd
from concourse.masks import make_identity

F32 = mybir.dt.float32
BF16 = mybir.dt.bfloat16
I32 = mybir.dt.int32
AF = mybir.ActivationFunctionType
ALU = mybir.AluOpType

B_TOT, S = 64, 512
DC = DB = 200
HID = H = 300
VC, VB = 10000, 200000
NCORES = 8
BL = B_TOT // NCORES          # 8 sentences per core
T = BL * S                    # 4096 tokens per core
G4 = 4 * H                    # 1200

# smoke-test overrides (break numerics, only to exercise compile/run quickly)
N_TILES = int(os.environ.get("K_NTILES", T // 128))   # 32
STEPS = int(os.environ.get("K_STEPS", S))             # 512
ABL = os.environ.get("K_ABL", "")          # ablation flags: hs,xdma,tp

# gate permutation: torch/ref order (i,f,g,o) -> kernel order (i,f,o,g)
_PERM = np.r_[0:300, 300:600, 900:1200, 600:900]

M300 = [128, 128, 44]         # chunks of 300 (lin output dims / recurrence h)
N512 = [(0, 512), (512, 512), (1024, 176)]  # free-dim chunks of 1200
KXP = [128, 128, 65]          # xproj contraction chunks (65 = 44 dims + ones@64)
RB = 32                       # right chain's partition base (32-aligned)


def _build_program():
    nc = bacc.Bacc()

    idx_d = nc.declare_dram_parameter("idx", [128, N_TILES * 8], I32, isOutput=False)
    tab_char = nc.declare_dram_parameter("char_embed", [VC, DC], F32, isOutput=False)
    tab_schar = nc.declare_dram_parameter("static_char_embed", [VC, DC], F32, isOutput=False)
    tab_bi = nc.declare_dram_parameter("bichar_embed", [VB, DB], F32, isOutput=False)
    tab_sbi = nc.declare_dram_parameter("static_bichar_embed", [VB, DB], F32, isOutput=False)
    wlin_d = nc.declare_dram_parameter("wlin_blk", [128, 24 * 128], BF16, isOutput=False)
    blin_d = nc.declare_dram_parameter("blin_blk", [128, 3], F32, isOutput=False)
    wih_d = nc.declare_dram_parameter("wihaug_blk", [128, 2 * 3 * G4], BF16, isOutput=False)
    whh12_d = nc.declare_dram_parameter("whh12_blk", [128, 2 * 2 * G4], BF16, isOutput=False)
    whh3_d = nc.declare_dram_parameter("whh3_blk", [44, 2 * G4], BF16, isOutput=False)
    i8_d = nc.declare_dram_parameter("i8blk", [8, 8], BF16, isOutput=False)
    ones_d = nc.declare_dram_parameter("onesblk", [1, 128], BF16, isOutput=False)
    hs_d = nc.declare_dram_parameter("hs", [2, T, H], F32, isOutput=True)
    x_d = nc.dram_tensor("x_seq", [2, T, G4], BF16)

    tables = [tab_char, tab_schar, tab_bi, tab_sbi]

    with tile.TileContext(nc) as tc:
        with (
            tc.tile_pool(name="const", bufs=1) as cp,
            tc.tile_pool(name="ph_sb", bufs=2) as pp,
            tc.tile_pool(name="rc_sb", bufs=2) as rp,
            tc.tile_pool(name="rc_h", bufs=4) as hp,
            tc.tile_pool(name="ps", bufs=1, space="PSUM") as psp,
        ):
            ident = cp.tile([128, 128], F32, tag="ident")
            make_identity(nc, ident[:, :])
            idx_sb = cp.tile([128, N_TILES * 8], I32, tag="idx")
            nc.sync.dma_start(out=idx_sb[:, :], in_=idx_d[:, :])
            wlin_sb = cp.tile([128, 24 * 128], BF16, tag="wlin")
            nc.sync.dma_start(out=wlin_sb[:, :], in_=wlin_d[:, :])
            blin_sb = cp.tile([128, 3], F32, tag="blin")
            nc.sync.dma_start(out=blin_sb[:, :], in_=blin_d[:, :])
            wih_sb = cp.tile([128, 2 * 3 * G4], BF16, tag="wih")
            nc.sync.dma_start(out=wih_sb[:, :], in_=wih_d[:, :])
            whh12_sb = cp.tile([128, 2 * 2 * G4], BF16, tag="whh12")
            nc.sync.dma_start(out=whh12_sb[:, :], in_=whh12_d[:, :])

            # persistent linT tiles (side x parity); ones row 64 loaded once
            linTs = {}
            for side in range(2):
                for par in range(2):
                    lt = cp.tile([128, 3 * 128], BF16, tag=f"linT_{side}_{par}")
                    nc.sync.dma_start(out=lt[64:65, 256:384], in_=ones_d[:, :])
                    linTs[(side, par)] = lt

            # recurrence state, fully per-chain to keep the two scans decoupled
            hT1s, hT2s, hT3s, c_sts, b3s = [], [], [], [], []
            for c in range(2):
                t1 = cp.tile([128, 8], BF16, tag=f"hT1_{c}")
                nc.vector.memset(t1[:, :], 0.0)
                t2 = cp.tile([128, 8], BF16, tag=f"hT2_{c}")
                nc.vector.memset(t2[:, :], 0.0)
                t3 = cp.tile([52, 8], BF16, tag=f"hT3_{c}")
                nc.vector.memset(t3[0:44, :], 0.0)
                nc.sync.dma_start(out=t3[44:52, 0:8], in_=i8_d[:, :])
                cs = cp.tile([8, H], F32, tag=f"c_{c}")
                nc.vector.memset(cs[:, :], 0.0)
                hT1s.append(t1); hT2s.append(t2); hT3s.append(t3); c_sts.append(cs)
                bufs = []
                for r in range(4):
                    b3 = cp.tile([52, G4], BF16, tag=f"b3_{c}_{r}")
                    nc.sync.dma_start(out=b3[0:44, :], in_=whh3_d[0:44, c * G4:(c + 1) * G4])
                    bufs.append(b3)
                b3s.append(bufs)

            # ---------------- phases 1-3: gather, transpose, linear, xproj ----
            for t in range(N_TILES):
                for side in range(2):
                    feat = pp.tile([128, 800], F32, tag=f"feat{side}")
                    for j4 in range(4):
                        col = t * 8 + side * 4 + j4
                        nc.gpsimd.indirect_dma_start(
                            out=feat[:, 200 * j4:200 * (j4 + 1)],
                            out_offset=None,
                            in_=tables[j4][:, :],
                            in_offset=bass.IndirectOffsetOnAxis(
                                ap=idx_sb[:, col:col + 1], axis=0),
                        )
                    # 8 transposes of 100-col slices (each inside one gather segment)
                    featT = pp.tile([128, 8 * 128], BF16, tag=f"ft{side}")
                    for kc in range(8):
                        tp = psp.tile([128, 128], F32, tag="plg1", bufs=2)
                        nc.tensor.transpose(
                            tp[0:100, 0:128], feat[:, kc * 100:(kc + 1) * 100],
                            ident[:, :])
                        nc.vector.tensor_copy(
                            featT[0:100, kc * 128:(kc + 1) * 128], tp[0:100, 0:128])
                    linT = linTs[(side, t % 2)]
                    for m in range(3):
                        mm = M300[m]
                        pl = psp.tile([128, 128], F32, tag="plg1", bufs=2)
                        for kc in range(8):
                            blk = (kc * 3 + m) * 128
                            nc.tensor.matmul(
                                pl[0:mm, 0:128],
                                lhsT=wlin_sb[0:100, blk:blk + mm],
                                rhs=featT[0:100, kc * 128:(kc + 1) * 128],
                                start=(kc == 0), stop=(kc == 7))
                        nc.scalar.activation(
                            linT[0:mm, m * 128:m * 128 + 128],
                            pl[0:mm, 0:128], AF.Tanh,
                            bias=blin_sb[0:mm, m:m + 1])
                    px = psp.tile([128, G4], F32, tag="pxg0", bufs=2)
                    for kc in range(3):
                        kw = KXP[kc]
                        for (n0, nw) in N512:
                            nc.tensor.matmul(
                                px[:, n0:n0 + nw],
                                lhsT=linT[0:kw, kc * 128:kc * 128 + 128],
                                rhs=wih_sb[0:kw, (side * 3 + kc) * G4 + n0:
                                           (side * 3 + kc) * G4 + n0 + nw],
                                start=(kc == 0), stop=(kc == 2))
                    x_sb = pp.tile([128, G4], BF16, tag=f"x{side}")
                    for (n0, nw) in N512:
                        nc.scalar.copy(x_sb[:, n0:n0 + nw], px[:, n0:n0 + nw])
                    nc.sync.dma_start(
                        out=x_d[side, t * 128:(t + 1) * 128, :], in_=x_sb[:, :])

            # ---------------- phase 4: the two LSTM scans ---------------------
            # Per-chain dependency chains (independent, interleaved on engines).
            # c' = f*c + 2*(i*s) - i  where s = sigmoid(2g)  [tanh-free g path]
            abl = set(ABL.split(","))
            for t in range(STEPS):
                for c in range(2):
                    pb = c * RB
                    b3 = b3s[c][t % 4]
                    if "xdma" not in abl:
                        nc.sync.dma_start(
                            out=b3[44:52, :], in_=x_d[c, t * 8:(t + 1) * 8, :])
                    ps = psp.tile([8, G4], F32, tag="pxg0", bufs=2)
                    tpos = None if c == 0 else (0, RB)
                    for (n0, nw) in N512:
                        nc.tensor.matmul(
                            ps[:, n0:n0 + nw],
                            lhsT=hT1s[c][:, 0:8],
                            rhs=whh12_sb[:, (c * 2) * G4 + n0:(c * 2) * G4 + n0 + nw],
                            start=True, stop=False)
                        nc.tensor.matmul(
                            ps[:, n0:n0 + nw],
                            lhsT=hT2s[c][:, 0:8],
                            rhs=whh12_sb[:, (c * 2 + 1) * G4 + n0:
                                         (c * 2 + 1) * G4 + n0 + nw],
                            start=False, stop=False)
                        nc.tensor.matmul(
                            ps[:, n0:n0 + nw],
                            lhsT=hT3s[c][0:52, 0:8],
                            rhs=b3[0:52, n0:n0 + nw], start=False, stop=True)
                    sg = rp.tile([8, G4], F32, tag=f"sg{c}")
                    nc.scalar.activation(sg[:, :], ps[:, :], AF.Sigmoid)
                    A = rp.tile([8, H], F32, tag=f"A{c}")
                    nc.vector.tensor_tensor(
                        A[:, :], sg[:, 0:300], sg[:, 900:1200], op=ALU.mult)
                    Dt = rp.tile([8, H], F32, tag=f"D{c}")
                    nc.vector.tensor_tensor(
                        Dt[:, :], sg[:, 300:600], c_sts[c][:, :], op=ALU.mult)
                    A2 = rp.tile([8, H], F32, tag=f"A2{c}")
                    nc.vector.tensor_tensor(A2[:, :], A[:, :], A[:, :], op=ALU.add)
                    D2 = rp.tile([8, H], F32, tag=f"D2{c}")
                    nc.vector.tensor_tensor(
                        D2[:, :], Dt[:, :], sg[:, 0:300], op=ALU.subtract)
                    nc.vector.tensor_tensor(
                        c_sts[c][:, :], A2[:, :], D2[:, :], op=ALU.add)
                    tc_t = rp.tile([8, H], F32, tag=f"tc{c}")
                    nc.scalar.activation(tc_t[:, :], c_sts[c][:, :], AF.Tanh)
                    h = hp.tile([8, H], F32, tag=f"h{c}")
                    nc.vector.tensor_tensor(
                        h[:, :], sg[:, 600:900], tc_t[:, :], op=ALU.mult)
                    if "hs" not in abl:
                        nc.sync.dma_start(
                            out=hs_d[c, t * 8:(t + 1) * 8, :], in_=h[:, :])
                    if "tp" in abl:
                        continue
                    idn = ident[0:8, 0:8]
                    tp1 = psp.tile([128, 8], F32, tag="plg1", bufs=2)
                    nc.tensor.transpose(tp1[:, 0:8], h[:, 0:128], idn)
                    nc.scalar.copy(hT1s[c][:, 0:8], tp1[:, 0:8])
                    tp2 = psp.tile([128, 8], F32, tag="plg1", bufs=2)
                    nc.tensor.transpose(tp2[:, 0:8], h[:, 128:256], idn)
                    nc.scalar.copy(hT2s[c][:, 0:8], tp2[:, 0:8])
                    tp3 = psp.tile([128, 8], F32, tag="plg1", bufs=2)
                    nc.tensor.transpose(tp3[0:44, 0:8], h[:, 256:300], idn)
                    nc.scalar.copy(hT3s[c][0:44, 0:8], tp3[0:44, 0:8])
    nc.compile()
    return nc


def _prep_host(inputs):
    """Build the per-core in_maps (host-side weight/index preprocessing)."""
    f = {k: np.asarray(v) for k, v in inputs.items()}

    wlinT = f["W_lin"].astype(np.float32).T            # [800, 300]
    wlin_blk = np.zeros((128, 24 * 128), np.float32)
    for kc in range(8):
        for m in range(3):
            mm = M300[m]
            blk = (kc * 3 + m) * 128
            wlin_blk[0:100, blk:blk + mm] = wlinT[kc * 100:(kc + 1) * 100,
                                                 m * 128:m * 128 + mm]
    blin_blk = np.zeros((128, 3), np.float32)
    for m in range(3):
        mm = M300[m]
        blin_blk[0:mm, m] = f["b_lin"][m * 128:m * 128 + mm]

    wih_blk = np.zeros((128, 2 * 3 * G4), np.float32)
    whh12_blk = np.zeros((128, 2 * 2 * G4), np.float32)
    whh3_blk = np.zeros((44, 2 * G4), np.float32)
    gscale = np.ones((G4,), np.float32)
    gscale[900:1200] = 2.0      # g-gate pre-activations doubled (sigmoid trick)
    for c, sfx in enumerate(("l", "r")):
        wihT = (f[f"Wih_{sfx}"][_PERM, :].astype(np.float32) * gscale[:, None]).T
        bb = f[f"b_{sfx}"][_PERM].astype(np.float32) * gscale
        for kc in range(2):
            wih_blk[0:128, (c * 3 + kc) * G4:(c * 3 + kc + 1) * G4] = \
                wihT[kc * 128:(kc + 1) * 128, :]
        wih_blk[0:44, (c * 3 + 2) * G4:(c * 3 + 3) * G4] = wihT[256:300, :]
        wih_blk[64, (c * 3 + 2) * G4:(c * 3 + 3) * G4] = bb
        whhT = (f[f"Whh_{sfx}"][_PERM, :].astype(np.float32) * gscale[:, None]).T
        whh12_blk[:, (c * 2) * G4:(c * 2 + 1) * G4] = whhT[0:128, :]
        whh12_blk[:, (c * 2 + 1) * G4:(c * 2 + 2) * G4] = whhT[128:256, :]
        whh3_blk[:, c * G4:(c + 1) * G4] = whhT[256:300, :]

    import ml_dtypes
    bf = lambda a: a.astype(ml_dtypes.bfloat16)
    shared = {
        "char_embed": f["char_embed"].astype(np.float32),
        "static_char_embed": f["static_char_embed"].astype(np.float32),
        "bichar_embed": f["bichar_embed"].astype(np.float32),
        "static_bichar_embed": f["static_bichar_embed"].astype(np.float32),
        "wlin_blk": bf(wlin_blk), "blin_blk": blin_blk,
        "wihaug_blk": bf(wih_blk), "whh12_blk": bf(whh12_blk),
        "whh3_blk": bf(whh3_blk),
        "i8blk": bf(np.eye(8, dtype=np.float32)),
        "onesblk": bf(np.ones((1, 128), np.float32)),
    }

    in_maps = []
    for core in range(NCORES):
        bs = slice(core * BL, (core + 1) * BL)
        idx_blk = np.zeros((128, N_TILES * 8), np.int32)
        # stream order: [charL scharL bilL sbilL | charR scharR birR sbirR]
        streams = [
            f["char_features"][bs].T.reshape(-1),
            f["static_char_features"][bs].T.reshape(-1),
            f["bichar_left_features"][bs].T.reshape(-1),
            f["static_bichar_left_features"][bs].T.reshape(-1),
            f["char_features"][bs][:, ::-1].T.reshape(-1),
            f["static_char_features"][bs][:, ::-1].T.reshape(-1),
            f["bichar_right_features"][bs][:, ::-1].T.reshape(-1),
            f["static_bichar_right_features"][bs][:, ::-1].T.reshape(-1),
        ]
        for t in range(N_TILES):
            for j in range(8):
                idx_blk[:, t * 8 + j] = streams[j][t * 128:(t + 1) * 128]
        in_maps.append({"idx": idx_blk, **shared})
    return in_maps


_CACHED = {}


def kernel(**inputs):
    if "nc" not in _CACHED:
        _CACHED["nc"] = _build_program()
    nc = _CACHED["nc"]
    in_maps = _prep_host(inputs)
    res = run_bass_kernel_spmd(nc, in_maps, list(range(NCORES)))
    _CACHED["res"] = res
    out = np.empty((B_TOT, S, 2 * H), np.float32)
    for core in range(NCORES):
        hs = res.results[core]["hs"].reshape(2, S, BL, H)
        bs = slice(core * BL, (core + 1) * BL)
        out[bs, :, 0:H] = hs[0].transpose(1, 0, 2)
        out[bs, :, H:2 * H] = hs[1, ::-1].transpose(1, 0, 2)
    return out


if __name__ == "__main__":
    sys.path.insert(0, os.path.dirname(os.path.abspath(__file__)))
    import reference
    inp = reference.setup_inputs()
    got = kernel(**{k: np.asarray(v) for k, v in inp.items()})
    exp = np.asarray(reference.reference(**inp))
    err = np.abs(got - exp)
    rel = err.max() / np.abs(exp).max()
    print("Relative error:", rel)



# revision 9
# speedup vs baseline: 1.3308x; 1.3308x over previous
"""Trainium2 Bass kernel for nn_Encoder_WordLstm (bi-LSTM over char/bichar embeddings).

Sharding: data-parallel over batch. Each of the 8 cores handles 8 sentences and
runs BOTH LSTM directions.

Scan layout (the serial bottleneck): the 1200 gate columns per side are split by
c-dim strip (0:128 padded to 172, 128:300) and re-ordered host-side into two
344-col PSUM banks per strip: bankA=[i|f], bankB=[g|o]. The four (side, strip)
matmul streams land on PE column-groups 0/32/64/96 and run concurrently; gates
for both strips and both sides share partition-spanning SBUF tiles so each
activation / elementwise instruction covers 4 row-strips at once while the two
side-chains stay independent (row slices 0:40 and 64:104). h/tanh(c) are bf16;
f*c runs on GPSIMD to unload the DVE. h is transposed back (row-tiled PE
transposes at partition bases 0/32/64/96) to feed the next step's lhsT.

Per-core pipeline:
  1. indirect-DMA gathers of 4 embedding streams x 2 sides -> feat [128tok, 800]
  2. PE transpose -> featT(bf16), matmul W_lin + tanh -> linT [300, 128tok] bf16
  3. matmul Wih (bias via ones-row) -> x tiles (bf16, padded gate layout) -> DRAM
  4. 512-step LSTM recurrence; right direction consumes pre-reversed gather
     indices so its scan is a plain forward loop.
Output hs [2, 512, 40, 172] bf16 per core; host reassembles [64, 512, 600].
"""

import os
import sys

import numpy as np

sys.path.insert(0, "/opt/trn_rl_repo")

import concourse.bass as bass
import concourse.bacc as bacc
import concourse.mybir as mybir
import concourse.tile as tile
from concourse.bass_utils import run_bass_kernel_spmd
from concourse.masks import make_identity

F32 = mybir.dt.float32
BF16 = mybir.dt.bfloat16
I32 = mybir.dt.int32
AF = mybir.ActivationFunctionType
ALU = mybir.AluOpType

B_TOT, S = 64, 512
DC = DB = 200
HID = H = 300
VC, VB = 10000, 200000
NCORES = 8
BL = B_TOT // NCORES          # 8 sentences per core
T = BL * S                    # 4096 tokens per core
G4 = 4 * H                    # 1200 true gate cols per side
GP = 1376                     # padded gate cols per side (4 banks x 344)
CW = 172                      # c-dims per strip slot (strip0 uses 128 + 44 pad)

N_TILES = T // 128            # 32

M300 = [128, 128, 44]         # chunks of 300 (lin output dims)
KXP = [128, 128, 65]          # xproj contraction chunks (65 = 44 dims + ones@64)
PXC = [(0, 688), (688, 688)]  # xproj psum half-tiles
# within a px half: bank-safe MM slices (f32 bank = 512 elems)
PXN = [(0, 512), (512, 176)]

# scan strip row bases: (side, strip) -> psum/sbuf partition base
RBASE = {(0, 0): 0, (0, 1): 32, (1, 0): 64, (1, 1): 96}
# gate-col range in the padded-1376 layout for (strip, bank)
GCOL = {(0, 0): 0, (0, 1): 344, (1, 0): 688, (1, 1): 1032}


def _build_program():
    nc = bacc.Bacc()

    idx_d = nc.declare_dram_parameter("idx", [128, N_TILES * 8], I32, isOutput=False)
    tab_char = nc.declare_dram_parameter("char_embed", [VC, DC], F32, isOutput=False)
    tab_schar = nc.declare_dram_parameter("static_char_embed", [VC, DC], F32, isOutput=False)
    tab_bi = nc.declare_dram_parameter("bichar_embed", [VB, DB], F32, isOutput=False)
    tab_sbi = nc.declare_dram_parameter("static_bichar_embed", [VB, DB], F32, isOutput=False)
    wlin_d = nc.declare_dram_parameter("wlin_blk", [128, 24 * 128], BF16, isOutput=False)
    blin_d = nc.declare_dram_parameter("blin_blk", [128, 3], F32, isOutput=False)
    wih_d = nc.declare_dram_parameter("wihaug_blk", [128, 2 * 3 * GP], BF16, isOutput=False)
    whh12_d = nc.declare_dram_parameter("whh12_blk", [128, 2 * 2 * GP], BF16, isOutput=False)
    whh3_d = nc.declare_dram_parameter("whh3_blk", [44, 2 * GP], BF16, isOutput=False)
    i8_d = nc.declare_dram_parameter("i8blk", [8, 8], BF16, isOutput=False)
    ones_d = nc.declare_dram_parameter("onesblk", [1, 128], BF16, isOutput=False)
    hs_d = nc.declare_dram_parameter("hs", [2, S, 40, CW], BF16, isOutput=True)
    x_d = nc.dram_tensor("x_seq", [2, T, GP], BF16)

    tables = [tab_char, tab_schar, tab_bi, tab_sbi]

    with tile.TileContext(nc) as tc:
        with (
            tc.tile_pool(name="const", bufs=1) as cp,
            tc.tile_pool(name="ph_sb", bufs=2) as pp,
            tc.tile_pool(name="rc_sb", bufs=2) as rp,
            tc.tile_pool(name="ps", bufs=1, space="PSUM") as psp,
        ):
            ident = cp.tile([128, 128], F32, tag="ident")
            make_identity(nc, ident[:, :])
            identb = cp.tile([128, 128], BF16, tag="identb")
            nc.scalar.copy(identb[:, :], ident[:, :])
            idx_sb = cp.tile([128, N_TILES * 8], I32, tag="idx")
            nc.sync.dma_start(out=idx_sb[:, :], in_=idx_d[:, :])
            wlin_sb = cp.tile([128, 24 * 128], BF16, tag="wlin")
            nc.sync.dma_start(out=wlin_sb[:, :], in_=wlin_d[:, :])
            blin_sb = cp.tile([128, 3], F32, tag="blin")
            nc.sync.dma_start(out=blin_sb[:, :], in_=blin_d[:, :])
            wih_sb = cp.tile([128, 2 * 3 * GP], BF16, tag="wih")
            nc.sync.dma_start(out=wih_sb[:, :], in_=wih_d[:, :])
            whh12_sb = cp.tile([128, 2 * 2 * GP], BF16, tag="whh12")
            nc.sync.dma_start(out=whh12_sb[:, :], in_=whh12_d[:, :])

            # persistent linT tiles (side x parity); ones row 64 loaded once
            linTs = {}
            for side in range(2):
                for par in range(2):
                    lt = cp.tile([128, 3 * 128], BF16, tag=f"linT_{side}_{par}")
                    nc.sync.dma_start(out=lt[64:65, 256:384], in_=ones_d[:, :])
                    linTs[(side, par)] = lt

            # recurrence state: hT lhsT tiles per side, c shared [104, CW]
            hT1s, hT2s, hT3s, b3s = [], [], [], []
            for c in range(2):
                t1 = cp.tile([128, 8], BF16, tag=f"hT1_{c}")
                nc.vector.memset(t1[:, :], 0.0)
                t2 = cp.tile([128, 8], BF16, tag=f"hT2_{c}")
                nc.vector.memset(t2[:, :], 0.0)
                t3 = cp.tile([52, 8], BF16, tag=f"hT3_{c}")
                nc.vector.memset(t3[0:44, :], 0.0)
                nc.sync.dma_start(out=t3[44:52, 0:8], in_=i8_d[:, :])
                hT1s.append(t1); hT2s.append(t2); hT3s.append(t3)
                bufs = []
                for r in range(4):
                    b3 = cp.tile([52, GP], BF16, tag=f"b3_{c}_{r}")
                    nc.sync.dma_start(out=b3[0:44, :], in_=whh3_d[0:44, c * GP:(c + 1) * GP])
                    bufs.append(b3)
                b3s.append(bufs)
            c_st = cp.tile([104, CW], F32, tag="c_st")
            nc.vector.memset(c_st[:, :], 0.0)

            # ---------------- phases 1-3: gather, transpose, linear, xproj ----
            for t in range(N_TILES):
                for side in range(2):
                    feat = pp.tile([128, 800], F32, tag=f"feat{side}")
                    for j4 in range(4):
                        col = t * 8 + side * 4 + j4
                        nc.gpsimd.indirect_dma_start(
                            out=feat[:, 200 * j4:200 * (j4 + 1)],
                            out_offset=None,
                            in_=tables[j4][:, :],
                            in_offset=bass.IndirectOffsetOnAxis(
                                ap=idx_sb[:, col:col + 1], axis=0),
                        )
                    # 8 transposes of 100-col slices (each inside one gather segment)
                    featT = pp.tile([128, 8 * 128], BF16, tag=f"ft{side}")
                    for kc in range(8):
                        tp = psp.tile([128, 128], F32, tag="ptp", bufs=2)
                        nc.tensor.transpose(
                            tp[0:100, 0:128], feat[:, kc * 100:(kc + 1) * 100],
                            ident[:, :])
                        nc.vector.tensor_copy(
                            featT[0:100, kc * 128:(kc + 1) * 128], tp[0:100, 0:128])
                    linT = linTs[(side, t % 2)]
                    for m in range(3):
                        mm = M300[m]
                        pl = psp.tile([128, 128], F32, tag="ptp", bufs=2)
                        for kc in range(8):
                            blk = (kc * 3 + m) * 128
                            nc.tensor.matmul(
                                pl[0:mm, 0:128],
                                lhsT=wlin_sb[0:100, blk:blk + mm],
                                rhs=featT[0:100, kc * 128:(kc + 1) * 128],
                                start=(kc == 0), stop=(kc == 7))
                        nc.scalar.activation(
                            linT[0:mm, m * 128:m * 128 + 128],
                            pl[0:mm, 0:128], AF.Tanh,
                            bias=blin_sb[0:mm, m:m + 1])
                    x_sb = pp.tile([128, GP], BF16, tag=f"x{side}")
                    for (h0, hw) in PXC:
                        px = psp.tile([128, 688], F32, tag="px", bufs=1)
                        for kc in range(3):
                            kw = KXP[kc]
                            for (n0, nw) in PXN:
                                nc.tensor.matmul(
                                    px[:, n0:n0 + nw],
                                    lhsT=linT[0:kw, kc * 128:kc * 128 + 128],
                                    rhs=wih_sb[0:kw, side * 3 * GP + kc * GP + h0 + n0:
                                               side * 3 * GP + kc * GP + h0 + n0 + nw],
                                    start=(kc == 0), stop=(kc == 2))
                        nc.scalar.copy(x_sb[:, h0:h0 + hw], px[:, 0:hw])
                    nc.sync.dma_start(
                        out=x_d[side, t * 128:(t + 1) * 128, :], in_=x_sb[:, :])

            # ---------------- phase 4: the two LSTM scans ---------------------
            # Per-side chains; gates for (side, strip) land on psum rows
            # 0:8 / 32:40 (side l) and 64:72 / 96:104 (side r) via col-tiling.
            for t in range(S):
                ga = psp.tile([104, 344], F32, tag="GA", bufs=2)
                gb = psp.tile([104, 344], F32, tag="GB", bufs=2)
                for side in range(2):
                    b3 = b3s[side][t % 4]
                    nc.sync.dma_start(
                        out=b3[44:52, :], in_=x_d[side, t * 8:(t + 1) * 8, :])
                    r0 = 64 * side
                    for strip in range(2):
                        rb = RBASE[(side, strip)]
                        for bank, gt_ in ((0, ga), (1, gb)):
                            col = GCOL[(strip, bank)]
                            out = gt_[rb:rb + 8, 0:344]
                            nc.tensor.matmul(
                                out, lhsT=hT1s[side][:, 0:8],
                                rhs=whh12_sb[:, (side * 2) * GP + col:
                                             (side * 2) * GP + col + 344],
                                start=True, stop=False, tile_position=(0, rb))
                            nc.tensor.matmul(
                                out, lhsT=hT2s[side][:, 0:8],
                                rhs=whh12_sb[:, (side * 2 + 1) * GP + col:
                                             (side * 2 + 1) * GP + col + 344],
                                start=False, stop=False, tile_position=(0, rb))
                            nc.tensor.matmul(
                                out, lhsT=hT3s[side][0:52, 0:8],
                                rhs=b3[0:52, col:col + 344],
                                start=False, stop=True, tile_position=(0, rb))
                    # activations: rows r0:r0+40 cover both strips of this side
                    sga = rp.tile([104, 344], F32, tag="sga")
                    nc.scalar.activation(
                        sga[r0:r0 + 40, :], ga[r0:r0 + 40, :], AF.Sigmoid)
                    gob = rp.tile([104, 344], F32, tag="gob")
                    nc.scalar.activation(
                        gob[r0:r0 + 40, 0:CW], gb[r0:r0 + 40, 0:CW], AF.Tanh)
                    nc.scalar.activation(
                        gob[r0:r0 + 40, CW:344], gb[r0:r0 + 40, CW:344], AF.Sigmoid)
                    # c' = f*c + i*g ; h = o*tanh(c')
                    pt = rp.tile([104, CW], F32, tag="pt")
                    nc.vector.tensor_tensor(
                        pt[r0:r0 + 40, :], sga[r0:r0 + 40, 0:CW],
                        gob[r0:r0 + 40, 0:CW], op=ALU.mult)
                    dt = rp.tile([104, CW], F32, tag="dt")
                    nc.gpsimd.tensor_tensor(
                        dt[r0:r0 + 40, :], sga[r0:r0 + 40, CW:344],
                        c_st[r0:r0 + 40, :], op=ALU.mult)
                    nc.vector.tensor_tensor(
                        c_st[r0:r0 + 40, :], pt[r0:r0 + 40, :],
                        dt[r0:r0 + 40, :], op=ALU.add)
                    tct = rp.tile([104, CW], F32, tag="tct")
                    nc.scalar.activation(
                        tct[r0:r0 + 40, :], c_st[r0:r0 + 40, :], AF.Tanh)
                    h = rp.tile([104, CW], BF16, tag="h")
                    nc.vector.tensor_tensor(
                        h[r0:r0 + 40, :], gob[r0:r0 + 40, CW:344],
                        tct[r0:r0 + 40, :], op=ALU.mult)
                    nc.sync.dma_start(
                        out=hs_d[side, t, :, :], in_=h[r0:r0 + 40, :])
                    # transposes back to lhsT layout for the next step
                    s0, s1 = r0, r0 + 32
                    tp1 = psp.tile([128, 8], BF16, tag="ptp", bufs=2)
                    nc.tensor.transpose(
                        tp1[0:128, 0:8], h[s0:s0 + 8, 0:128],
                        identb[s0:s0 + 8, s0:s0 + 8], tile_position=(s0, 0))
                    nc.scalar.copy(hT1s[side][:, 0:8], tp1[0:128, 0:8])
                    tp2 = psp.tile([128, 8], BF16, tag="ptp", bufs=2)
                    nc.tensor.transpose(
                        tp2[0:128, 0:8], h[s1:s1 + 8, 0:128],
                        identb[s1:s1 + 8, s1:s1 + 8], tile_position=(s1, 0))
                    nc.scalar.copy(hT2s[side][:, 0:8], tp2[0:128, 0:8])
                    tp3 = psp.tile([128, 8], BF16, tag="ptp", bufs=2)
                    nc.tensor.transpose(
                        tp3[0:44, 0:8], h[s1:s1 + 8, 128:172],
                        identb[s1:s1 + 8, s1:s1 + 8], tile_position=(s1, 0))
                    nc.scalar.copy(hT3s[side][0:44, 0:8], tp3[0:44, 0:8])
    nc.compile()
    return nc


def _gate_perm():
    """Column permutation: padded-1376 col -> source col in reference (i,f,g,o)
    order, or -1 for a zero pad column."""
    perm = np.full(GP, -1, np.int64)
    goff = {"i": 0, "f": 300, "g": 600, "o": 900}
    order = {0: ("i", "f"), 1: ("g", "o")}
    for strip in range(2):
        lo, hi = (0, 128) if strip == 0 else (128, 300)
        for bank in range(2):
            base = GCOL[(strip, bank)]
            for slot, gname in enumerate(order[bank]):
                dst = base + slot * CW
                perm[dst:dst + (hi - lo)] = goff[gname] + np.arange(lo, hi)
    return perm


def _prep_host(inputs):
    """Build the per-core in_maps (host-side weight/index preprocessing)."""
    f = {k: np.asarray(v) for k, v in inputs.items()}

    wlinT = f["W_lin"].astype(np.float32).T            # [800, 300]
    wlin_blk = np.zeros((128, 24 * 128), np.float32)
    for kc in range(8):
        for m in range(3):
            mm = M300[m]
            blk = (kc * 3 + m) * 128
            wlin_blk[0:100, blk:blk + mm] = wlinT[kc * 100:(kc + 1) * 100,
                                                 m * 128:m * 128 + mm]
    blin_blk = np.zeros((128, 3), np.float32)
    for m in range(3):
        mm = M300[m]
        blin_blk[0:mm, m] = f["b_lin"][m * 128:m * 128 + mm]

    perm = _gate_perm()
    valid = perm >= 0

    wih_blk = np.zeros((128, 2 * 3 * GP), np.float32)
    whh12_blk = np.zeros((128, 2 * 2 * GP), np.float32)
    whh3_blk = np.zeros((44, 2 * GP), np.float32)
    for c, sfx in enumerate(("l", "r")):
        wihT = f[f"Wih_{sfx}"].astype(np.float32).T     # [300, 1200]
        bb = f[f"b_{sfx}"].astype(np.float32)           # [1200]
        wihP = np.zeros((300, GP), np.float32)
        wihP[:, valid] = wihT[:, perm[valid]]
        bbP = np.zeros(GP, np.float32)
        bbP[valid] = bb[perm[valid]]
        for kc in range(2):
            wih_blk[0:128, (c * 3 + kc) * GP:(c * 3 + kc + 1) * GP] = \
                wihP[kc * 128:(kc + 1) * 128, :]
        wih_blk[0:44, (c * 3 + 2) * GP:(c * 3 + 3) * GP] = wihP[256:300, :]
        wih_blk[64, (c * 3 + 2) * GP:(c * 3 + 3) * GP] = bbP
        whhT = f[f"Whh_{sfx}"].astype(np.float32).T     # [300, 1200]
        whhP = np.zeros((300, GP), np.float32)
        whhP[:, valid] = whhT[:, perm[valid]]
        whh12_blk[:, (c * 2) * GP:(c * 2 + 1) * GP] = whhP[0:128, :]
        whh12_blk[:, (c * 2 + 1) * GP:(c * 2 + 2) * GP] = whhP[128:256, :]
        whh3_blk[:, c * GP:(c + 1) * GP] = whhP[256:300, :]

    import ml_dtypes
    bf = lambda a: a.astype(ml_dtypes.bfloat16)
    shared = {
        "char_embed": f["char_embed"].astype(np.float32),
        "static_char_embed": f["static_char_embed"].astype(np.float32),
        "bichar_embed": f["bichar_embed"].astype(np.float32),
        "static_bichar_embed": f["static_bichar_embed"].astype(np.float32),
        "wlin_blk": bf(wlin_blk), "blin_blk": blin_blk,
        "wihaug_blk": bf(wih_blk), "whh12_blk": bf(whh12_blk),
        "whh3_blk": bf(whh3_blk),
        "i8blk": bf(np.eye(8, dtype=np.float32)),
        "onesblk": bf(np.ones((1, 128), np.float32)),
    }

    in_maps = []
    for core in range(NCORES):
        bs = slice(core * BL, (core + 1) * BL)
        idx_blk = np.zeros((128, N_TILES * 8), np.int32)
        # stream order: [charL scharL bilL sbilL | charR scharR birR sbirR]
        streams = [
            f["char_features"][bs].T.reshape(-1),
            f["static_char_features"][bs].T.reshape(-1),
            f["bichar_left_features"][bs].T.reshape(-1),
            f["static_bichar_left_features"][bs].T.reshape(-1),
            f["char_features"][bs][:, ::-1].T.reshape(-1),
            f["static_char_features"][bs][:, ::-1].T.reshape(-1),
            f["bichar_right_features"][bs][:, ::-1].T.reshape(-1),
            f["static_bichar_right_features"][bs][:, ::-1].T.reshape(-1),
        ]
        for t in range(N_TILES):
            for j in range(8):
                idx_blk[:, t * 8 + j] = streams[j][t * 128:(t + 1) * 128]
        in_maps.append({"idx": idx_blk, **shared})
    return in_maps


_CACHED = {}


def kernel(**inputs):
    if "nc" not in _CACHED:
        _CACHED["nc"] = _build_program()
    nc = _CACHED["nc"]
    in_maps = _prep_host(inputs)
    res = run_bass_kernel_spmd(nc, in_maps, list(range(NCORES)))
    _CACHED["res"] = res
    out = np.empty((B_TOT, S, 2 * H), np.float32)
    for core in range(NCORES):
        hs = res.results[core]["hs"].astype(np.float32)   # [2, S, 40, CW]
        bs = slice(core * BL, (core + 1) * BL)
        for side in range(2):
            hfull = np.empty((S, BL, H), np.float32)
            hfull[:, :, 0:128] = hs[side, :, 0:8, 0:128]
            hfull[:, :, 128:300] = hs[side, :, 32:40, 0:172]
            if side == 1:
                hfull = hfull[::-1]
            out[bs, :, side * H:(side + 1) * H] = hfull.transpose(1, 0, 2)
    return out


if __name__ == "__main__":
    sys.path.insert(0, os.path.dirname(os.path.abspath(__file__)))
    import reference
    inp = reference.setup_inputs()
    got = kernel(**{k: np.asarray(v) for k, v in inp.items()})
    exp = np.asarray(reference.reference(**inp))
    err = np.abs(got - exp)
    rel = err.max() / np.abs(exp).max()
    print("Relative error:", rel)


# revision 25
# speedup vs baseline: 1.4400x; 1.0821x over previous
"""Trainium2 Bass kernel for nn_Encoder_WordLstm (bi-LSTM over char/bichar embeddings).

Sharding: data-parallel over batch. Each of the 8 cores handles 8 sentences and
runs BOTH LSTM directions.

Scan layout (the serial bottleneck): the 1200 gate columns per side are split by
c-dim strip (0:128 padded to 172, 128:300) and re-ordered host-side into two
344-col PSUM banks per strip: bankA=[i|f], bankB=[g|o]. The four (side, strip)
matmul streams land on PE column-groups 0/32/64/96 and run concurrently; gates
for both strips and both sides share partition-spanning SBUF tiles so each
activation / elementwise instruction covers 4 row-strips at once while the two
side-chains stay independent (row slices 0:40 and 64:104). h/tanh(c) are bf16;
f*c runs on GPSIMD to unload the DVE. h is transposed back (row-tiled PE
transposes at partition bases 0/32/64/96) to feed the next step's lhsT.

Per-core pipeline:
  1. indirect-DMA gathers of 4 embedding streams x 2 sides -> feat [128tok, 800]
  2. PE transpose -> featT(bf16), matmul W_lin + tanh -> linT [300, 128tok] bf16
  3. matmul Wih (bias via ones-row) -> x tiles (bf16, padded gate layout) -> DRAM
  4. 512-step LSTM recurrence; right direction consumes pre-reversed gather
     indices so its scan is a plain forward loop.
Output hs [2, 512, 40, 172] bf16 per core; host reassembles [64, 512, 600].
"""

import os
import sys

import numpy as np

sys.path.insert(0, "/opt/trn_rl_repo")

import concourse.bass as bass
import concourse.bacc as bacc
import concourse.mybir as mybir
import concourse.tile as tile
from concourse.bass_utils import run_bass_kernel_spmd
from concourse.masks import make_identity

F32 = mybir.dt.float32
BF16 = mybir.dt.bfloat16
I32 = mybir.dt.int32
AF = mybir.ActivationFunctionType
ALU = mybir.AluOpType

B_TOT, S = 64, 512
DC = DB = 200
HID = H = 300
VC, VB = 10000, 200000
NCORES = 8
BL = B_TOT // NCORES          # 8 sentences per core
T = BL * S                    # 4096 tokens per core
G4 = 4 * H                    # 1200 true gate cols per side
GP = 1376                     # padded gate cols per side (4 banks x 344)
CW = 172                      # c-dims per strip slot (strip0 uses 128 + 44 pad)

N_TILES = T // 128            # 32

M300 = [128, 128, 44]         # chunks of 300 (lin output dims)
KXP = [128, 128, 65]          # xproj contraction chunks (65 = 44 dims + ones@64)


# scan strip row bases: (side, strip) -> psum/sbuf partition base
RBASE = {(0, 0): 0, (0, 1): 32, (1, 0): 64, (1, 1): 96}
# gate-col range in the padded-1376 layout for (strip, bank)
GCOL = {(0, 0): 0, (0, 1): 344, (1, 0): 688, (1, 1): 1032}


def _build_program():
    nc = bacc.Bacc()

    idx_d = nc.declare_dram_parameter("idx", [128, N_TILES * 8], I32, isOutput=False)
    tab_char = nc.declare_dram_parameter("char_embed", [VC, DC], F32, isOutput=False)
    tab_schar = nc.declare_dram_parameter("static_char_embed", [VC, DC], F32, isOutput=False)
    tab_bi = nc.declare_dram_parameter("bichar_embed", [VB, DB], F32, isOutput=False)
    tab_sbi = nc.declare_dram_parameter("static_bichar_embed", [VB, DB], F32, isOutput=False)
    wlin_d = nc.declare_dram_parameter("wlin_blk", [128, 24 * 128], BF16, isOutput=False)
    blin_d = nc.declare_dram_parameter("blin_blk", [128, 3], F32, isOutput=False)
    wih_d = nc.declare_dram_parameter("wihaug_blk", [128, 2 * 3 * GP], BF16, isOutput=False)
    whh12_d = nc.declare_dram_parameter("whh12_blk", [128, 2 * 2 * GP], BF16, isOutput=False)
    whh3_d = nc.declare_dram_parameter("whh3_blk", [44, 2 * GP], BF16, isOutput=False)
    i8_d = nc.declare_dram_parameter("i8blk", [8, 8], BF16, isOutput=False)
    ones_d = nc.declare_dram_parameter("onesblk", [1, 128], BF16, isOutput=False)
    hs_d = nc.declare_dram_parameter("hs", [2, S, 40, CW], BF16, isOutput=True)
    x_d = nc.dram_tensor("x_seq", [2, T, GP], BF16)

    tables = [tab_char, tab_schar, tab_bi, tab_sbi]

    with tile.TileContext(nc) as tc:
        with (
            tc.tile_pool(name="const", bufs=1) as cp,
            tc.tile_pool(name="ph_sb", bufs=2) as pp,
            tc.tile_pool(name="rc_sb", bufs=2) as rp,
            tc.tile_pool(name="ps", bufs=1, space="PSUM") as psp,
        ):
            ident = cp.tile([128, 128], F32, tag="ident")
            make_identity(nc, ident[:, :])
            identb = cp.tile([128, 128], BF16, tag="identb")
            nc.scalar.copy(identb[:, :], ident[:, :])
            idx_sb = cp.tile([128, N_TILES * 8], I32, tag="idx")
            nc.sync.dma_start(out=idx_sb[:, :], in_=idx_d[:, :])
            wlin_sb = cp.tile([128, 24 * 128], BF16, tag="wlin")
            nc.sync.dma_start(out=wlin_sb[:, :], in_=wlin_d[:, :])
            blin_sb = cp.tile([128, 3], F32, tag="blin")
            nc.sync.dma_start(out=blin_sb[:, :], in_=blin_d[:, :])
            wih_sb = cp.tile([128, 2 * 3 * GP], BF16, tag="wih")
            nc.sync.dma_start(out=wih_sb[:, :], in_=wih_d[:, :])
            whh12_sb = cp.tile([128, 2 * 2 * GP], BF16, tag="whh12")
            nc.sync.dma_start(out=whh12_sb[:, :], in_=whh12_d[:, :])

            # persistent linT tiles (side x parity); ones row 64 loaded once
            linTs = {}
            for side in range(2):
                for par in range(2):
                    lt = cp.tile([128, 3 * 128], BF16, tag=f"linT_{side}_{par}")
                    nc.sync.dma_start(out=lt[64:65, 256:384], in_=ones_d[:, :])
                    linTs[(side, par)] = lt

            # recurrence state: hT lhsT tiles per side, c shared [104, CW]
            hT12s, hT3s, b3s = [], [], []
            for c in range(2):
                t12 = cp.tile([128, 16], BF16, tag=f"hT12_{c}")
                nc.vector.memset(t12[:, :], 0.0)
                t3 = cp.tile([52, 8], BF16, tag=f"hT3_{c}")
                nc.vector.memset(t3[0:44, :], 0.0)
                nc.sync.dma_start(out=t3[44:52, 0:8], in_=i8_d[:, :])
                hT12s.append(t12); hT3s.append(t3)
                bufs = []
                for r in range(8):
                    b3 = cp.tile([52, GP], BF16, tag=f"b3_{c}_{r}")
                    nc.sync.dma_start(out=b3[0:44, :], in_=whh3_d[0:44, c * GP:(c + 1) * GP])
                    bufs.append(b3)
                b3s.append(bufs)
            c_st = cp.tile([104, CW], F32, tag="c_st")
            nc.vector.memset(c_st[:, :], 0.0)

            # ---------------- phases 1-3: gather, transpose, linear, xproj ----
            # Emitted as a function so phase tiles can interleave with scan
            # steps: they fill PE idle slots during the scan's dependency
            # stalls (keeps the HAM clock-gate warm).
            def emit_phase_tile(t):
                for side in range(2):
                    feat = pp.tile([128, 800], F32, tag=f"feat{side}")
                    for j4 in range(4):
                        col = t * 8 + side * 4 + j4
                        nc.gpsimd.indirect_dma_start(
                            out=feat[:, 200 * j4:200 * (j4 + 1)],
                            out_offset=None,
                            in_=tables[j4][:, :],
                            in_offset=bass.IndirectOffsetOnAxis(
                                ap=idx_sb[:, col:col + 1], axis=0),
                        )
                    # 8 transposes of 100-col slices (each inside one gather segment)
                    featT = pp.tile([128, 8 * 128], BF16, tag=f"ft{side}")
                    for kc in range(8):
                        tp = psp.tile([128, 128], F32, tag="px", bufs=2)
                        nc.tensor.transpose(
                            tp[0:100, 0:128], feat[:, kc * 100:(kc + 1) * 100],
                            ident[:, :])
                        nc.vector.tensor_copy(
                            featT[0:100, kc * 128:(kc + 1) * 128], tp[0:100, 0:128])
                    linT = linTs[(side, t % 2)]
                    for m in range(3):
                        mm = M300[m]
                        pl = psp.tile([128, 128], F32, tag="px", bufs=2)
                        for kc in range(8):
                            blk = (kc * 3 + m) * 128
                            nc.tensor.matmul(
                                pl[0:mm, 0:128],
                                lhsT=wlin_sb[0:100, blk:blk + mm],
                                rhs=featT[0:100, kc * 128:(kc + 1) * 128],
                                start=(kc == 0), stop=(kc == 7))
                        nc.scalar.activation(
                            linT[0:mm, m * 128:m * 128 + 128],
                            pl[0:mm, 0:128], AF.Tanh,
                            bias=blin_sb[0:mm, m:m + 1])
                    x_sb = pp.tile([128, GP], BF16, tag=f"x{side}")
                    for q in range(4):
                        px = psp.tile([128, 344], F32, tag="px", bufs=2)
                        for kc in range(3):
                            kw = KXP[kc]
                            nc.tensor.matmul(
                                px[:, 0:344],
                                lhsT=linT[0:kw, kc * 128:kc * 128 + 128],
                                rhs=wih_sb[0:kw, side * 3 * GP + kc * GP + q * 344:
                                           side * 3 * GP + kc * GP + q * 344 + 344],
                                start=(kc == 0), stop=(kc == 2))
                        nc.scalar.copy(x_sb[:, q * 344:(q + 1) * 344], px[:, 0:344])
                    nc.sync.dma_start(
                        out=x_d[side, t * 128:(t + 1) * 128, :], in_=x_sb[:, :])

            # ---------------- phase 4: the two LSTM scans ---------------------
            # Per-side chains; gates for (side, strip) land on psum rows
            # 0:8 / 32:40 (side l) and 64:72 / 96:104 (side r) via col-tiling.
            def emit_scan_step(t):
                ga = psp.tile([104, 344], F32, tag="GA", bufs=2)
                gb = psp.tile([104, 344], F32, tag="GB", bufs=2)
                for side in range(2):
                    b3 = b3s[side][t % 8]
                    nc.sync.dma_start(
                        out=b3[44:52, :], in_=x_d[side, t * 8:(t + 1) * 8, :])
                    r0 = 64 * side
                    for bank, gt_ in ((0, ga), (1, gb)):
                        for strip in range(2):
                            rb = RBASE[(side, strip)]
                            col = GCOL[(strip, bank)]
                            out = gt_[rb:rb + 8, 0:344]
                            nc.tensor.matmul(
                                out, lhsT=hT12s[side][:, 0:8],
                                rhs=whh12_sb[:, (side * 2) * GP + col:
                                             (side * 2) * GP + col + 344],
                                start=True, stop=False, tile_position=(0, rb))
                            nc.tensor.matmul(
                                out, lhsT=hT12s[side][:, 8:16],
                                rhs=whh12_sb[:, (side * 2 + 1) * GP + col:
                                             (side * 2 + 1) * GP + col + 344],
                                start=False, stop=False, tile_position=(0, rb))
                            nc.tensor.matmul(
                                out, lhsT=hT3s[side][0:52, 0:8],
                                rhs=b3[0:52, col:col + 344],
                                start=False, stop=True, tile_position=(0, rb))
                    # activations: rows r0:r0+40 cover both strips of this side.
                    # sigA + D=f*c run while the bankB matmuls still stream.
                    sga = rp.tile([104, 344], F32, tag="sga")
                    nc.scalar.activation(
                        sga[r0:r0 + 40, :], ga[r0:r0 + 40, :], AF.Sigmoid)
                    dt = rp.tile([104, CW], F32, tag="dt")
                    nc.vector.tensor_tensor(
                        dt[r0:r0 + 40, :], sga[r0:r0 + 40, CW:344],
                        c_st[r0:r0 + 40, :], op=ALU.mult)
                    gob = rp.tile([104, 344], F32, tag="gob")
                    nc.scalar.activation(
                        gob[r0:r0 + 40, 0:CW], gb[r0:r0 + 40, 0:CW], AF.Tanh)
                    nc.scalar.activation(
                        gob[r0:r0 + 40, CW:344], gb[r0:r0 + 40, CW:344], AF.Sigmoid)
                    # c' = f*c + i*g ; h = o*tanh(c')
                    pt = rp.tile([104, CW], F32, tag="pt")
                    nc.vector.tensor_tensor(
                        pt[r0:r0 + 40, :], sga[r0:r0 + 40, 0:CW],
                        gob[r0:r0 + 40, 0:CW], op=ALU.mult)
                    nc.vector.tensor_tensor(
                        c_st[r0:r0 + 40, :], pt[r0:r0 + 40, :],
                        dt[r0:r0 + 40, :], op=ALU.add)
                    tct = rp.tile([104, CW], F32, tag="tct")
                    nc.scalar.activation(
                        tct[r0:r0 + 40, :], c_st[r0:r0 + 40, :], AF.Tanh)
                    h = rp.tile([104, CW], BF16, tag="h")
                    nc.vector.tensor_tensor(
                        h[r0:r0 + 40, :], gob[r0:r0 + 40, CW:344],
                        tct[r0:r0 + 40, :], op=ALU.mult)
                    nc.sync.dma_start(
                        out=hs_d[side, t, :, :], in_=h[r0:r0 + 40, :])
                    # transposes back to lhsT layout for the next step
                    s0, s1 = r0, r0 + 32
                    tp1 = psp.tile([128, 8], BF16, tag="ptp", bufs=2)
                    nc.tensor.transpose(
                        tp1[0:128, 0:8], h[s0:s0 + 8, 0:128],
                        identb[s0:s0 + 8, s0:s0 + 8], tile_position=(s0, 0))
                    nc.scalar.copy(hT12s[side][:, 0:8], tp1[0:128, 0:8])
                    tp2 = psp.tile([128, 8], BF16, tag="ptp", bufs=2)
                    nc.tensor.transpose(
                        tp2[0:128, 0:8], h[s1:s1 + 8, 0:128],
                        identb[s1:s1 + 8, s1:s1 + 8], tile_position=(s1, 0))
                    nc.scalar.copy(hT12s[side][:, 8:16], tp2[0:128, 0:8])
                    tp3 = psp.tile([128, 8], BF16, tag="ptp", bufs=2)
                    nc.tensor.transpose(
                        tp3[0:44, 0:8], h[s1:s1 + 8, 128:172],
                        identb[s1:s1 + 8, s1:s1 + 8], tile_position=(s1, 0))
                    nc.scalar.copy(hT3s[side][0:44, 0:8], tp3[0:44, 0:8])

            # interleaved driver: phase tile k covers scan steps 16k..16k+16;
            # stay 3 tiles (48 steps) ahead of the scan's x consumption
            LEAD = 3
            for k in range(LEAD):
                emit_phase_tile(k)
            for t in range(S):
                if t % 16 == 0 and t // 16 + LEAD < N_TILES:
                    emit_phase_tile(t // 16 + LEAD)
                emit_scan_step(t)
    nc.compile()
    return nc


def _gate_perm():
    """Column permutation: padded-1376 col -> source col in reference (i,f,g,o)
    order, or -1 for a zero pad column."""
    perm = np.full(GP, -1, np.int64)
    goff = {"i": 0, "f": 300, "g": 600, "o": 900}
    order = {0: ("i", "f"), 1: ("g", "o")}
    for strip in range(2):
        lo, hi = (0, 128) if strip == 0 else (128, 300)
        for bank in range(2):
            base = GCOL[(strip, bank)]
            for slot, gname in enumerate(order[bank]):
                dst = base + slot * CW
                perm[dst:dst + (hi - lo)] = goff[gname] + np.arange(lo, hi)
    return perm


def _prep_host(inputs):
    """Build the per-core in_maps (host-side weight/index preprocessing)."""
    f = {k: np.asarray(v) for k, v in inputs.items()}

    wlinT = f["W_lin"].astype(np.float32).T            # [800, 300]
    wlin_blk = np.zeros((128, 24 * 128), np.float32)
    for kc in range(8):
        for m in range(3):
            mm = M300[m]
            blk = (kc * 3 + m) * 128
            wlin_blk[0:100, blk:blk + mm] = wlinT[kc * 100:(kc + 1) * 100,
                                                 m * 128:m * 128 + mm]
    blin_blk = np.zeros((128, 3), np.float32)
    for m in range(3):
        mm = M300[m]
        blin_blk[0:mm, m] = f["b_lin"][m * 128:m * 128 + mm]

    perm = _gate_perm()
    valid = perm >= 0

    wih_blk = np.zeros((128, 2 * 3 * GP), np.float32)
    whh12_blk = np.zeros((128, 2 * 2 * GP), np.float32)
    whh3_blk = np.zeros((44, 2 * GP), np.float32)
    for c, sfx in enumerate(("l", "r")):
        wihT = f[f"Wih_{sfx}"].astype(np.float32).T     # [300, 1200]
        bb = f[f"b_{sfx}"].astype(np.float32)           # [1200]
        wihP = np.zeros((300, GP), np.float32)
        wihP[:, valid] = wihT[:, perm[valid]]
        bbP = np.zeros(GP, np.float32)
        bbP[valid] = bb[perm[valid]]
        for kc in range(2):
            wih_blk[0:128, (c * 3 + kc) * GP:(c * 3 + kc + 1) * GP] = \
                wihP[kc * 128:(kc + 1) * 128, :]
        wih_blk[0:44, (c * 3 + 2) * GP:(c * 3 + 3) * GP] = wihP[256:300, :]
        wih_blk[64, (c * 3 + 2) * GP:(c * 3 + 3) * GP] = bbP
        whhT = f[f"Whh_{sfx}"].astype(np.float32).T     # [300, 1200]
        whhP = np.zeros((300, GP), np.float32)
        whhP[:, valid] = whhT[:, perm[valid]]
        whh12_blk[:, (c * 2) * GP:(c * 2 + 1) * GP] = whhP[0:128, :]
        whh12_blk[:, (c * 2 + 1) * GP:(c * 2 + 2) * GP] = whhP[128:256, :]
        whh3_blk[:, c * GP:(c + 1) * GP] = whhP[256:300, :]

    import ml_dtypes
    bf = lambda a: a.astype(ml_dtypes.bfloat16)
    shared = {
        "char_embed": f["char_embed"].astype(np.float32),
        "static_char_embed": f["static_char_embed"].astype(np.float32),
        "bichar_embed": f["bichar_embed"].astype(np.float32),
        "static_bichar_embed": f["static_bichar_embed"].astype(np.float32),
        "wlin_blk": bf(wlin_blk), "blin_blk": blin_blk,
        "wihaug_blk": bf(wih_blk), "whh12_blk": bf(whh12_blk),
        "whh3_blk": bf(whh3_blk),
        "i8blk": bf(np.eye(8, dtype=np.float32)),
        "onesblk": bf(np.ones((1, 128), np.float32)),
    }

    in_maps = []
    for core in range(NCORES):
        bs = slice(core * BL, (core + 1) * BL)
        idx_blk = np.zeros((128, N_TILES * 8), np.int32)
        # stream order: [charL scharL bilL sbilL | charR scharR birR sbirR]
        streams = [
            f["char_features"][bs].T.reshape(-1),
            f["static_char_features"][bs].T.reshape(-1),
            f["bichar_left_features"][bs].T.reshape(-1),
            f["static_bichar_left_features"][bs].T.reshape(-1),
            f["char_features"][bs][:, ::-1].T.reshape(-1),
            f["static_char_features"][bs][:, ::-1].T.reshape(-1),
            f["bichar_right_features"][bs][:, ::-1].T.reshape(-1),
            f["static_bichar_right_features"][bs][:, ::-1].T.reshape(-1),
        ]
        for t in range(N_TILES):
            for j in range(8):
                idx_blk[:, t * 8 + j] = streams[j][t * 128:(t + 1) * 128]
        in_maps.append({"idx": idx_blk, **shared})
    return in_maps


_CACHED = {}


def kernel(**inputs):
    if "nc" not in _CACHED:
        _CACHED["nc"] = _build_program()
    nc = _CACHED["nc"]
    in_maps = _prep_host(inputs)
    res = run_bass_kernel_spmd(nc, in_maps, list(range(NCORES)))
    _CACHED["res"] = res
    out = np.empty((B_TOT, S, 2 * H), np.float32)
    for core in range(NCORES):
        hs = res.results[core]["hs"].astype(np.float32)   # [2, S, 40, CW]
        bs = slice(core * BL, (core + 1) * BL)
        for side in range(2):
            hfull = np.empty((S, BL, H), np.float32)
            hfull[:, :, 0:128] = hs[side, :, 0:8, 0:128]
            hfull[:, :, 128:300] = hs[side, :, 32:40, 0:172]
            if side == 1:
                hfull = hfull[::-1]
            out[bs, :, side * H:(side + 1) * H] = hfull.transpose(1, 0, 2)
    return out


if __name__ == "__main__":
    sys.path.insert(0, os.path.dirname(os.path.abspath(__file__)))
    import reference
    inp = reference.setup_inputs()
    got = kernel(**{k: np.asarray(v) for k, v in inp.items()})
    exp = np.asarray(reference.reference(**inp))
    err = np.abs(got - exp)
    rel = err.max() / np.abs(exp).max()
    print("Relative error:", rel)
